# revision 12
# baseline (speedup 1.0000x reference)
"""Trainium2 Bass kernel for nn_GResBlock (2-layer weighted-GCN residual block).

    h1 = relu(A @ x @ W1 + x @ W1_loop + b1)
    h2 = relu(A @ h1 @ W2 + h1 @ W2_loop + b2)
    out = (x + h2) * 0.5
(A = 50000^2 sparse adjacency given as an 800000-edge weighted list.)

Strategy (8 NeuronCores, SPMD — one program, per-core data):
- Vertices padded to 50176 = 8*6272 rows; core c owns dst nodes
  [c*6272, (c+1)*6272) split into 98 chunks of 64. Edges are bucketed by
  dst core, sorted by dst chunk, and split by src < 32768 (lo) / >= (hi)
  so int16 dma_gather indices stay in range (hi calls use a shifted base).
- Aggregation is reordered as (A @ x) @ W (associativity), so the gather
  table for layer 1 is x itself (bf16, rows padded to 256B).
- Per chunk, each 128-edge block is one PE matmul: stationary = gathered
  src rows [128, 96], moving = a block-sparse selector S [128, 64]
  (edge weights at the edge's dst lane), accumulating agg^T [96, 64] f32
  in PSUM. S is built ON-CHIP per chunk by two DVE ops (iota==lane)*w
  from compact resident lane/weight tables — no dense S in HBM.
- Gather indices live in two resident SBUF tables (one DMA each); each
  1024-index dma_gather call slices them. Calls round-robin 4 SWDGE queues.
- psum2 = Wloop_aug^T @ src_aug (bias via ones row) + W^T @ agg -> relu.
  Layer-1 sources (x^T) and the layer-2 residual (x^T/2) are SBUF-resident.
- Layer 1 tail: PE-transpose each h1^T chunk -> packed h1 rows [6272, 96]
  -> one AllGather (wire 9.6MB bf16 or 4.8MB fp8) -> on-device expansion
  into the 256B-strided layer-2 gather table.
- Output returned transposed per core and re-assembled on the host.
"""
import os
import sys

import numpy as np
import ml_dtypes

try:
    import concourse.bass  # noqa: F401
except ImportError:
    sys.path.insert(0, "/opt/trn_rl_repo")

import concourse.bass as bass  # noqa: E402
import concourse.tile as tile  # noqa: E402
from concourse.ap import AP  # noqa: E402
from concourse.tile_rust import add_dep_helper  # noqa: E402
from concourse import bacc, mybir  # noqa: E402
from concourse.library_config import mlp  # noqa: E402
from concourse.bass_utils import run_bass_kernel_spmd  # noqa: E402

bf16 = ml_dtypes.bfloat16
f8 = ml_dtypes.float8_e4m3fn
BF16 = mybir.dt.bfloat16
F8 = mybir.dt.float8e4
F32 = mybir.dt.float32
I16 = mybir.dt.int16

N_NODES = 50000
D = 96
NC = 8
SHARD = 6272
NPAD = NC * SHARD          # 50176
CHUNK = 64
NCHUNK = SHARD // CHUNK    # 98
HALF = 32768
ELEM = 128                 # gather element width (bf16 -> 256B)
NQ = 4                     # SWDGE queues (ucode max)
CALL_BLK = 8               # 128-edge blocks per gather call
CALL_IDX = 1024            # indices per gather call (hw scratch cap)
EXP_K = 28                 # expansion rows-per-partition (14 x 128*28 = NPAD)

AG_MODE = os.environ.get("GK_AG", "pack")   # pack | full | fp8


def _preprocess(edge_src, edge_dst, edge_weight):
    src = np.asarray(edge_src).astype(np.int64)
    dst = np.asarray(edge_dst).astype(np.int64)
    w = np.asarray(edge_weight).astype(np.float32)
    E = src.shape[0]

    g = dst // CHUNK                       # global chunk id 0..NC*NCHUNK-1
    hi = (src >= HALF).astype(np.int64)
    seg = g * 2 + hi
    order = np.argsort(seg, kind="stable")
    seg_s = seg[order]
    src_s = src[order]
    w_s = w[order]
    lane_s = (dst % CHUNK)[order]

    nseg = 2 * NC * NCHUNK
    cnt = np.bincount(seg_s, minlength=nseg)
    starts = np.concatenate([[0], np.cumsum(cnt)])
    pos = np.arange(E) - starts[seg_s]
    blk = pos >> 7
    lib = pos & 127

    Bseg = -(-cnt // 128)
    B_lo = max(1, int(Bseg[0::2].max()))
    B_hi = max(1, int(Bseg[1::2].max()))
    NB = B_lo + B_hi
    NCALL_LO = -(-(NCHUNK * B_lo) // CALL_BLK)
    NCALL_HI = -(-(NCHUNK * B_hi) // CALL_BLK)

    cores = seg_s // (2 * NCHUNK)
    k_all = (seg_s // 2) % NCHUNK
    ishi = seg_s & 1

    def wrap_all(a, ncall):
        a = a.reshape(ncall, CALL_IDX // 16, 16).astype(np.int16)
        a = np.tile(a.transpose(0, 2, 1), (1, 8, 1))           # [ncall,128,64]
        return np.ascontiguousarray(
            a.transpose(1, 0, 2).reshape(128, ncall * (CALL_IDX // 16)))

    out = []
    for c in range(NC):
        m = cores == c
        k = k_all[m]
        h = ishi[m]
        s = src_s[m]
        ww = w_s[m]
        ll = lane_s[m]
        bb = blk[m]
        li = lib[m]

        lo = h == 0
        idx_lo = np.zeros(NCALL_LO * CALL_IDX, np.int64)
        slot = (k[lo] * B_lo + bb[lo]) * 128 + li[lo]
        idx_lo[slot] = s[lo]
        idx_hi = np.zeros(NCALL_HI * CALL_IDX, np.int64)
        him = ~lo
        slot = (k[him] * B_hi + bb[him]) * 128 + li[him]
        idx_hi[slot] = s[him] - HALF

        lanes = np.zeros((128, NCHUNK * NB), np.float32)
        wv = np.zeros((128, NCHUNK * NB), np.float32)
        col = k * NB + np.where(h == 1, B_lo + bb, bb)
        lanes[li, col] = ll
        wv[li, col] = ww
        out.append(dict(
            idx_lo=wrap_all(idx_lo, NCALL_LO),
            idx_hi=wrap_all(idx_hi, NCALL_HI),
            lanes=lanes.astype(bf16),
            wv=wv.astype(bf16),
        ))
    return out, B_lo, B_hi


def _make_in_maps(x, W1, W1_loop, b1, W2, W2_loop, b2, edge_weight, edge_src, edge_dst):
    pp, B_lo, B_hi = _preprocess(edge_src, edge_dst, edge_weight)
    x = np.asarray(x, np.float32)
    xtab = np.zeros((NPAD, ELEM), bf16)
    xtab[:N_NODES, :D] = x.astype(bf16)
    xpad = np.zeros((NPAD, D), np.float32)
    xpad[:N_NODES] = x
    W1a = np.concatenate([np.asarray(W1_loop, np.float32),
                          np.asarray(b1, np.float32)[None, :]], 0).astype(bf16)
    W2a = np.concatenate([np.asarray(W2_loop, np.float32),
                          np.asarray(b2, np.float32)[None, :]], 0).astype(bf16)
    in_maps = []
    for c in range(NC):
        xs = xpad[c * SHARD:(c + 1) * SHARD]
        xT_aug = np.ones((D + 1, SHARD), bf16)
        xT_aug[:D] = xs.T.astype(bf16)
        in_maps.append(dict(
            xtab=xtab,
            xT_aug=xT_aug,
            xT_half=np.ascontiguousarray(0.5 * xs.T).astype(np.float32),
            W1=np.asarray(W1, np.float32).astype(bf16),
            W2=np.asarray(W2, np.float32).astype(bf16),
            W1a=W1a, W2a=W2a,
            lanes=pp[c]["lanes"],
            wv=pp[c]["wv"],
            idx_lo=pp[c]["idx_lo"],
            idx_hi=pp[c]["idx_hi"],
        ))
    return in_maps, B_lo, B_hi


def build_program(B_lo, B_hi, repeat=0, ag_reps=1, parts="all", ag_mode=None):
    """Build the SPMD Bass program. repeat>0 wraps each gconv phase in a
    hardware For_i loop and emits the AllGather ag_reps times (timing only;
    collectives cannot sit inside hardware loops)."""
    ag_mode = ag_mode or AG_MODE
    NB = B_lo + B_hi
    NCALL_LO = -(-(NCHUNK * B_lo) // CALL_BLK)
    NCALL_HI = -(-(NCHUNK * B_hi) // CALL_BLK)
    TBL_DT = F8 if ag_mode == "fp8" else BF16
    nc = bacc.Bacc("TRN2", target_bir_lowering=False, debug=False, num_devices=NC,
                   num_swdge_queues=NQ)

    xtab = nc.dram_tensor("xtab", [NPAD, ELEM], BF16, kind="ExternalInput")
    xT_aug = nc.dram_tensor("xT_aug", [D + 1, SHARD], BF16, kind="ExternalInput")
    xT_half = nc.dram_tensor("xT_half", [D, SHARD], F32, kind="ExternalInput")
    W1 = nc.dram_tensor("W1", [D, D], BF16, kind="ExternalInput")
    W2 = nc.dram_tensor("W2", [D, D], BF16, kind="ExternalInput")
    W1a = nc.dram_tensor("W1a", [D + 1, D], BF16, kind="ExternalInput")
    W2a = nc.dram_tensor("W2a", [D + 1, D], BF16, kind="ExternalInput")
    lanes_d = nc.dram_tensor("lanes", [128, NCHUNK * NB], BF16, kind="ExternalInput")
    wv_d = nc.dram_tensor("wv", [128, NCHUNK * NB], BF16, kind="ExternalInput")
    idx_lo_d = nc.dram_tensor("idx_lo", [128, NCALL_LO * (CALL_IDX // 16)], I16,
                              kind="ExternalInput")
    idx_hi_d = nc.dram_tensor("idx_hi", [128, NCALL_HI * (CALL_IDX // 16)], I16,
                              kind="ExternalInput")
    outT = nc.dram_tensor("outT", [D, SHARD], F32, kind="ExternalOutput")

    with tile.TileContext(nc) as tc:
        from contextlib import ExitStack
        with ExitStack() as ctx:
            const = ctx.enter_context(tc.tile_pool(name="const", bufs=1))
            mlop = ctx.enter_context(tc.tile_pool(name="mlop", bufs=8))
            mhip = ctx.enter_context(tc.tile_pool(name="mhip", bufs=5))
            sp = ctx.enter_context(tc.tile_pool(name="sp", bufs=3))
            mkp = ctx.enter_context(tc.tile_pool(name="mkp", bufs=3))
            aggsbp = ctx.enter_context(tc.tile_pool(name="aggsbp", bufs=3))
            rowp = ctx.enter_context(tc.tile_pool(name="rowp", bufs=3))
            outp = ctx.enter_context(tc.tile_pool(name="outp", bufs=3))
            expp = ctx.enter_context(tc.tile_pool(name="expp", bufs=2))
            aggps = ctx.enter_context(tc.tile_pool(name="aggps", bufs=3, space="PSUM"))
            p2ps = ctx.enter_context(tc.tile_pool(name="p2ps", bufs=2, space="PSUM"))
            trps = ctx.enter_context(tc.tile_pool(name="trps", bufs=2, space="PSUM"))

            nc.gpsimd.load_library(mlp)

            ident_d = nc.inline_tensor(np.eye(D, dtype=bf16), name="ident_bf16")
            ident = const.tile([D, D], BF16)
            nc.sync.dma_start(ident[:], ident_d.ap())
            iota_np = np.tile(np.arange(CHUNK, dtype=np.float32), (128, 1)).astype(bf16)
            iota_d = nc.inline_tensor(iota_np, name="iota_bf16")
            iota_t = const.tile([128, CHUNK], BF16)
            nc.sync.dma_start(iota_t[:], iota_d.ap())
            w1 = const.tile([D, D], BF16)
            nc.sync.dma_start(w1[:], W1.ap())
            w2 = const.tile([D, D], BF16)
            nc.sync.dma_start(w2[:], W2.ap())
            w1a = const.tile([D + 1, D], BF16)
            nc.sync.dma_start(w1a[:], W1a.ap())
            w2a = const.tile([D + 1, D], BF16)
            nc.sync.dma_start(w2a[:], W2a.ap())
            lanes_t = const.tile([128, NCHUNK * NB], BF16)
            nc.sync.dma_start(lanes_t[:], lanes_d.ap())
            wv_t = const.tile([128, NCHUNK * NB], BF16)
            nc.sync.dma_start(wv_t[:], wv_d.ap())
            idxlo_t = const.tile([128, NCALL_LO * (CALL_IDX // 16)], I16)
            nc.sync.dma_start(idxlo_t[:], idx_lo_d.ap())
            idxhi_t = const.tile([128, NCALL_HI * (CALL_IDX // 16)], I16)
            nc.sync.dma_start(idxhi_t[:], idx_hi_d.ap())
            xTa_t = const.tile([D + 1, SHARD], BF16)
            nc.sync.dma_start(xTa_t[:], xT_aug.ap())
            xTh_t = const.tile([D, SHARD], F32)
            nc.sync.dma_start(xTh_t[:], xT_half.ap())

            h1t = const.tile([D + 1, SHARD], BF16)   # persistent h1^T (+ones row)
            nc.vector.memset(h1t[D:D + 1, :], 1.0)

            state = {"gq": 0, "prev_gather": None}
            if ag_mode == "full":
                h1_local = nc.dram_tensor("h1_local", [SHARD, ELEM], BF16,
                                          kind="Internal").ap()
                h1_table = nc.dram_tensor("h1_table", [NPAD, ELEM], BF16,
                                          kind="Internal", addr_space="Shared").ap()
                h1p = h1tp = None
            else:
                h1p = nc.dram_tensor("h1p", [SHARD, D], TBL_DT, kind="Internal").ap()
                h1tp = nc.dram_tensor("h1tp", [NPAD, D], TBL_DT, kind="Internal",
                                      addr_space="Shared").ap()
                h1_table = nc.dram_tensor("h1_table", [NPAD, ELEM], BF16,
                                          kind="Internal").ap()
                h1_local = None

            def bcast3(ap2, inner):
                """[P, n] AP -> [P, n, inner] with broadcast innermost dim."""
                return AP(ap2.tensor, ap2.offset, [ap2.ap[0], ap2.ap[1], [0, inner]])

            def group3(ap2, n, inner):
                """[P, n*inner] contiguous AP -> [P, n, inner]."""
                return AP(ap2.tensor, ap2.offset, [ap2.ap[0], [inner, n], [1, inner]])

            i_ap = iota_t[:]
            iota3 = AP(i_ap.tensor, i_ap.offset, [i_ap.ap[0], [0, NB], i_ap.ap[1]])

            def gconv(layer, table_ap, w_t, wa_t):
                lo_tiles = {}
                hi_tiles = {}

                def emit_call(tiles, idx_t, c, half):
                    m = (mlop if half == 0 else mhip).tile(
                        [128, CALL_BLK, ELEM], BF16, tag="m")
                    if parts in ("nogather", "nos"):
                        nc.vector.memset(m[:, 0:1, :], 0.0)
                        tiles[c] = m
                        return
                    base = table_ap[0:HALF, :] if half == 0 else table_ap[HALF:NPAD, :]
                    gi = nc.gpsimd.dma_gather(
                        m[:], base, idx_t[:, c * 64:(c + 1) * 64],
                        CALL_IDX, CALL_IDX, ELEM, queue_num=state["gq"] % NQ)
                    state["gq"] += 1
                    if (state["prev_gather"] is not None
                            and os.environ.get("GK_NODEP", "0") != "1"):
                        # Keep Pool-engine order = emission order so Tile's
                        # 8-lane DMASW sem rotation stays aligned with the
                        # 4-queue rotation (sems are queue-locked).
                        add_dep_helper(gi.ins, state["prev_gather"].ins, sync=False,
                                       reason="swdge queue/sem-lane consistency")
                    state["prev_gather"] = gi
                    tiles[c] = m

                SG = 4                      # chunks per S-build op pair
                sgroup = {}

                for k in range(NCHUNK):
                    for j in range(B_lo):
                        c = (k * B_lo + j) // CALL_BLK
                        if c not in lo_tiles:
                            emit_call(lo_tiles, idxlo_t, c, 0)
                    for j in range(B_hi):
                        c = (k * B_hi + j) // CALL_BLK
                        if c not in hi_tiles:
                            emit_call(hi_tiles, idxhi_t, c, 1)
                    if parts in ("gather", "g1"):
                        continue
                    # --- S build on-chip: S[p, b*64+l] = w[p,b] * (l == lane[p,b])
                    kg = k // SG
                    if kg not in sgroup:
                        ng = min(SG, NCHUNK - kg * SG) * NB
                        stg = sp.tile([128, SG * NB * CHUNK], BF16, tag="st")
                        if parts == "nos":
                            nc.vector.memset(stg[:, 0:1], 0.0)
                        else:
                            k0 = kg * SG
                            mk = mkp.tile([128, SG * NB * CHUNK], BF16, tag="mk")
                            iota3g = AP(i_ap.tensor, i_ap.offset,
                                        [i_ap.ap[0], [0, ng], i_ap.ap[1]])
                            nc.vector.tensor_tensor(
                                group3(mk[:], ng, CHUNK), iota3g,
                                bcast3(lanes_t[:, k0 * NB:k0 * NB + ng], CHUNK),
                                mybir.AluOpType.is_equal)
                            nc.vector.tensor_tensor(
                                group3(stg[:], ng, CHUNK), group3(mk[:], ng, CHUNK),
                                bcast3(wv_t[:, k0 * NB:k0 * NB + ng], CHUNK),
                                mybir.AluOpType.mult)
                        sgroup[kg] = stg
                    st = sgroup[kg][:, (k % SG) * NB * CHUNK:
                                    ((k % SG) + 1) * NB * CHUNK]
                    agg = aggps.tile([D, CHUNK], F32, tag="agg")
                    for j in range(B_lo):
                        b = k * B_lo + j
                        nc.tensor.matmul(
                            agg[:], lo_tiles[b // CALL_BLK][:, b % CALL_BLK, 0:D],
                            st[:, j * CHUNK:(j + 1) * CHUNK],
                            start=(j == 0), stop=False, skip_group_check=True)
                    for j in range(B_hi):
                        b = k * B_hi + j
                        nc.tensor.matmul(
                            agg[:], hi_tiles[b // CALL_BLK][:, b % CALL_BLK, 0:D],
                            st[:, (B_lo + j) * CHUNK:(B_lo + j + 1) * CHUNK],
                            start=False, stop=(j == B_hi - 1), skip_group_check=True)
                    aggb = aggsbp.tile([D, CHUNK], BF16, tag="aggb")
                    nc.scalar.activation(aggb[:], agg[:],
                                         mybir.ActivationFunctionType.Copy)
                    p2 = p2ps.tile([D, CHUNK], F32, tag="p2")
                    srcap = (xTa_t if layer == 1 else h1t)[:, k * CHUNK:(k + 1) * CHUNK]
                    nc.tensor.matmul(p2[:], wa_t[:], srcap,
                                     start=True, stop=False, skip_group_check=True)
                    nc.tensor.matmul(p2[:], w_t[:], aggb[:],
                                     start=False, stop=True, skip_group_check=True)
                    if layer == 1:
                        hslice = h1t[0:D, k * CHUNK:(k + 1) * CHUNK]
                        nc.scalar.activation(hslice, p2[:],
                                             mybir.ActivationFunctionType.Relu)
                        trp = trps.tile([CHUNK, D], BF16, tag="trp")
                        nc.tensor.transpose(trp[:], hslice, ident[:])
                        row = rowp.tile([CHUNK, D], TBL_DT if ag_mode != "full"
                                        else BF16, tag="row")
                        nc.vector.tensor_copy(row[:], trp[:])
                        if ag_mode == "full":
                            nc.sync.dma_start(
                                h1_local[k * CHUNK:(k + 1) * CHUNK, 0:D], row[:])
                        else:
                            nc.sync.dma_start(
                                h1p[k * CHUNK:(k + 1) * CHUNK, :], row[:])
                    else:
                        relu = outp.tile([D, CHUNK], F32, tag="relu")
                        nc.scalar.activation(relu[:], p2[:],
                                             mybir.ActivationFunctionType.Relu,
                                             scale=0.5)
                        ot = outp.tile([D, CHUNK], F32, tag="ot")
                        nc.vector.tensor_add(ot[:], relu[:],
                                             xTh_t[:, k * CHUNK:(k + 1) * CHUNK])
                        nc.sync.dma_start(outT.ap()[:, k * CHUNK:(k + 1) * CHUNK], ot[:])

            if parts == "gather":
                zt = outp.tile([D, SHARD], F32, tag="zt")
                nc.vector.memset(zt[:], 0.0)
                nc.sync.dma_start(outT.ap(), zt[:])

            AG_FLAT = os.environ.get("GK_AGFLAT", "1") == "1"

            def flat(ap2, n):
                return AP(ap2.tensor, 0, [[1, n]])

            def allgather():
                if ag_mode == "full":
                    i_, o_ = h1_local[:], h1_table[:]
                    if AG_FLAT:
                        i_, o_ = flat(i_, SHARD * ELEM), flat(o_, NPAD * ELEM)
                else:
                    i_, o_ = h1p[:], h1tp[:]
                    if AG_FLAT:
                        i_, o_ = flat(i_, SHARD * D), flat(o_, NPAD * D)
                nc.gpsimd.collective_compute(
                    "AllGather", mybir.AluOpType.bypass,
                    ins=[i_], outs=[o_],
                    replica_groups=[list(range(NC))],
                )

            def expand():
                """h1tp [NPAD, 96] packed -> h1_table [NPAD, 128] 256B-strided.
                Loads and stores are fully contiguous (pad cols carry garbage,
                never read); the restride happens on DVE inside SBUF."""
                if ag_mode == "full":
                    return
                in_dt = F8 if ag_mode == "fp8" else BF16
                for t in range(NPAD // (128 * EXP_K)):
                    src_off = t * 128 * EXP_K * D
                    dst_off = t * 128 * EXP_K * ELEM
                    src_ap = AP(h1tp.tensor, src_off,
                                [[D * EXP_K, 128], [1, D * EXP_K]])
                    dst_ap = AP(h1_table.tensor, dst_off,
                                [[ELEM * EXP_K, 128], [1, ELEM * EXP_K]])
                    ei = expp.tile([128, EXP_K * D], in_dt, tag="ei")
                    nc.sync.dma_start(ei[:], src_ap)
                    eo = expp.tile([128, EXP_K * ELEM], BF16, tag="eo")
                    eo_ap = AP(eo[:].tensor, eo[:].offset,
                               [eo[:].ap[0], [ELEM, EXP_K], [1, D]])
                    nc.vector.tensor_copy(eo_ap, group3(ei[:], EXP_K, D))
                    nc.sync.dma_start(dst_ap, eo[:])

            l2_table = h1_table if ag_mode != "full" else h1_table

            if repeat > 0 and parts == "g1":
                with tc.For_i(0, repeat, 1):
                    gconv(1, xtab.ap(), w1, w1a)
                zt = outp.tile([D, SHARD], F32, tag="zt2")
                nc.vector.memset(zt[:], 0.0)
                nc.sync.dma_start(outT.ap(), zt[:])
            elif repeat > 0:
                with tc.For_i(0, repeat, 1):
                    gconv(1, xtab.ap(), w1, w1a)
                state["prev_gather"] = None
                for _ in range(ag_reps):
                    allgather()
                    expand()
                with tc.For_i(0, repeat, 1):
                    gconv(2, l2_table[:], w2, w2a)
            else:
                gconv(1, xtab.ap(), w1, w1a)
                allgather()
                expand()
                gconv(2, l2_table[:], w2, w2a)

    nc.compile()
    return nc


_CACHE = {}


def kernel(**inputs):
    in_maps, B_lo, B_hi = _make_in_maps(**inputs)
    key = (B_lo, B_hi, AG_MODE)
    if key not in _CACHE:
        _CACHE[key] = build_program(B_lo, B_hi)
    nc = _CACHE[key]
    r = run_bass_kernel_spmd(nc, in_maps, list(range(NC)))
    out = np.concatenate([r.results[c]["outT"].T for c in range(NC)], 0)[:N_NODES]
    return np.ascontiguousarray(out.astype(np.float32))
